# revision 1
# baseline (speedup 1.0000x reference)
"""MoE (top-2 of 8 experts) Trainium2 kernel — F-split expert sharding.

Instead of one expert per core (load-imbalanced: every core pays the max
expert load), every core processes ALL routed (token, expert) pairs but
only a 512-wide slice of the hidden dim F: core c owns columns
[c*512, (c+1)*512) of w1 and the matching rows of w2. Per-core PE work is
then exactly total_work/8 regardless of routing imbalance. Partial
y-contributions (the F-slice partial sums of layer 2) are summed on the
host during the combine, which is linear so the result is unchanged.

All matmul operands are bf16 (PSUM accumulation stays fp32): same PE
rate as f32r but half the DMA/SBUF footprint and FWL fast weight loads.
Adds ~3e-3 relative error - well within tolerance.

Layout per core (E=8 expert phases, FCL=4 f-blocks of 128 per phase):
  xT  [DC=8, 128, CAPT] bf16 - routed token stream, expert-major,
      identical on every core (CAPT = sum of 4-padded expert loads)
  w1p [E, 128, FCL*DC*128] bf16 - stationary tiles (fc,dc): [d x f]
  w2p [E, 128, DC*FCL*128] bf16 - stationary tiles (dc,fc): [f x d]
  b1p [128, E*FCL] f32
  yT  [DC=8, 128, CAPT] bf16 - partial y (this core's F-slice share)

Hardcoded problem shape: B=4, S=1024, D=1024, F=4096, E=8, TOP_K=2.
"""

import numpy as np

import concourse.bass as bass
import concourse.mybir as mybir
import concourse.tile as tile
from concourse import bacc
from concourse.bass_utils import run_bass_kernel_spmd

B, S, D, F, E = 4, 1024, 1024, 4096, 8
TOP_K = 2
P = 128
DC = D // P          # 8 d-blocks
FCL = F // E // P    # 4 f-blocks per core (F-slice = 512)

_program_cache: dict = {}


def _split(n, max_piece=512):
    """Split n into <=max_piece chunks, preferring equal large pieces."""
    k = -(-n // max_piece)
    base = n // k
    pieces = [base] * k
    rem = n - base * k
    for i in range(rem):
        pieces[i] += 1
    # keep chunk starts 4-aligned for clean DMA/PSUM addressing
    out, acc = [], 0
    for p in pieces[:-1]:
        p4 = (p // 4) * 4
        out.append(p4)
        acc += p4
    out.append(n - acc)
    return out


def _build_program(pes, reps=1, x_bufs=2, w_bufs=2, h_bufs=2, y_bufs=2,
                   psh_bufs=4, psy_bufs=4, max_chunk=512):
    """One SPMD program, identical on all cores.

    pes: per-expert padded token counts (the routed-pair stream layout).
    reps>1 repeats the whole computation (benchmarking only)."""
    capt = sum(pes)
    pemax = max(pes)
    bf16 = mybir.dt.bfloat16
    f32 = mybir.dt.float32

    nc = bacc.Bacc("TRN2", target_bir_lowering=False, debug=False, num_devices=E)
    xT = nc.dram_tensor("xT", [DC, P, capt], bf16, kind="ExternalInput")
    w1p = nc.dram_tensor("w1p", [E, P, FCL * D], bf16, kind="ExternalInput")
    w2p = nc.dram_tensor("w2p", [E, P, DC * FCL * P], bf16, kind="ExternalInput")
    b1p = nc.dram_tensor("b1p", [P, E * FCL], f32, kind="ExternalInput")
    yT = nc.dram_tensor("yT", [DC, P, capt], bf16, kind="ExternalOutput")

    with tile.TileContext(nc) as tc:
        with (
            tc.tile_pool(name="consts", bufs=1) as consts,
            tc.tile_pool(name="xp", bufs=x_bufs) as xp,
            tc.tile_pool(name="hp", bufs=h_bufs) as hp,
            tc.tile_pool(name="w1pool", bufs=w_bufs) as w1pool,
            tc.tile_pool(name="w2pool", bufs=w_bufs) as w2pool,
            tc.tile_pool(name="yp", bufs=y_bufs) as yp,
            tc.tile_pool(name="psh", bufs=psh_bufs, space="PSUM") as psh,
            tc.tile_pool(name="psy", bufs=psy_bufs, space="PSUM") as psy,
        ):
            b1_sb = consts.tile([P, E * FCL], f32)
            nc.sync.dma_start(b1_sb[:], b1p[:])

            offs = np.concatenate([[0], np.cumsum(pes)]).astype(int)
            phases = [(e, int(offs[e])) for _ in range(reps) for e in range(E)]

            def fetch(e):
                """Issue phase-e input DMAs (sync HWDGE queue only)."""
                oe = int(offs[e])
                pe = pes[e]
                x_sb = xp.tile([P, DC, pemax], bf16, tag="x")
                nc.sync.dma_start(
                    x_sb[:, :, :pe],
                    xT[:, :, oe:oe + pe].rearrange("d p t -> p d t"))
                w1_sb = w1pool.tile([P, FCL * D], bf16, tag="w1")
                nc.sync.dma_start(w1_sb[:], w1p[e])
                w2_sb = w2pool.tile([P, DC * FCL * P], bf16, tag="w2")
                nc.sync.dma_start(w2_sb[:], w2p[e])
                return x_sb, w1_sb, w2_sb

            fetched = fetch(phases[0][0])
            for i, (e, oe) in enumerate(phases):
                pe = pes[e]
                chunks = _split(pe, max_chunk)
                x_sb, w1_sb, w2_sb = fetched

                h_sb = hp.tile([P, FCL, pemax], bf16, tag="h")

                # layer 1: h[fc] = relu(sum_dc w1[fc,dc].T @ x[dc] + b1)
                for fc in range(FCL):
                    c0 = 0
                    for csz in chunks:
                        ph = psh.tile([P, max_chunk], f32, tag="ph")
                        for dc in range(DC):
                            nc.tensor.matmul(
                                ph[:, :csz],
                                w1_sb[:, (fc * DC + dc) * P:(fc * DC + dc + 1) * P],
                                x_sb[:, dc, c0:c0 + csz],
                                start=(dc == 0), stop=(dc == DC - 1),
                            )
                        nc.scalar.activation(
                            h_sb[:, fc, c0:c0 + csz], ph[:, :csz],
                            mybir.ActivationFunctionType.Relu,
                            bias=b1_sb[:, e * FCL + fc:e * FCL + fc + 1],
                        )
                        c0 += csz

                # prefetch next phase's inputs before emitting this phase's
                # layer-2 + y-out, so the sync DMA queue never has a y-out
                # blocking the next x/w transfer
                if i + 1 < len(phases):
                    fetched = fetch(phases[i + 1][0])

                # layer 2: y[dc] += sum_fc w2[dc,fc].T @ h[fc]  (partial)
                y_sb = yp.tile([P, DC, pemax], bf16, tag="y")
                for dc in range(DC):
                    c0 = 0
                    for csz in chunks:
                        py = psy.tile([P, max_chunk], f32, tag="py")
                        for fc in range(FCL):
                            nc.tensor.matmul(
                                py[:, :csz],
                                w2_sb[:, (dc * FCL + fc) * P:(dc * FCL + fc + 1) * P],
                                h_sb[:, fc, c0:c0 + csz],
                                start=(fc == 0), stop=(fc == FCL - 1),
                            )
                        nc.vector.tensor_copy(
                            y_sb[:, dc, c0:c0 + csz], py[:, :csz])
                        c0 += csz
                nc.scalar.dma_start(
                    yT[:, :, oe:oe + pe].rearrange("d p t -> p d t"),
                    y_sb[:, :, :pe])
    nc.finalize()
    return nc


def _route(x2d, gate_w, gate_b):
    """Host gate: softmax top-2 routing. Returns per-expert index lists and
    combine weights."""
    logits = (x2d @ gate_w + gate_b).astype(np.float64)
    logits -= logits.max(axis=-1, keepdims=True)
    p = np.exp(logits)
    p /= p.sum(axis=-1, keepdims=True)
    order = np.argsort(-p, axis=-1)[:, :TOP_K]
    idx = []
    cw = []
    for e in range(E):
        sel = np.nonzero((order == e).any(axis=-1))[0]
        idx.append(sel)
        cw.append(p[sel, e].astype(np.float32))
    return idx, cw


def _pad4(n):
    return max(4, -(-n // 4) * 4)


def _pack_inputs(x2d, idx, w1, b1, w2):
    """Build per-core input maps. x is identical on all cores; weights are
    the core's F-slice."""
    import ml_dtypes
    bf16 = ml_dtypes.bfloat16
    pes = [_pad4(len(i)) for i in idx]
    capt = sum(pes)

    xcat = np.zeros((capt, D), np.float32)
    oe = 0
    for e in range(E):
        xcat[oe:oe + len(idx[e])] = x2d[idx[e]]
        oe += pes[e]
    xT = np.ascontiguousarray(xcat.T.reshape(DC, P, capt).astype(bf16))

    in_maps = []
    for c in range(E):
        lo, hi = c * FCL * P, (c + 1) * FCL * P
        # w1p[e][p][(fc*DC+dc)*128+q] = w1[e][dc*128+p, lo+fc*128+q]
        w1p = np.stack([
            np.ascontiguousarray(
                w1[e][:, lo:hi].reshape(DC, P, FCL, P)
                .transpose(1, 2, 0, 3).reshape(P, FCL * D))
            for e in range(E)])
        # w2p[e][p][(dc*FCL+fc)*128+q] = w2[e][lo+fc*128+p, dc*128+q]
        w2p = np.stack([
            np.ascontiguousarray(
                w2[e][lo:hi, :].reshape(FCL, P, DC, P)
                .transpose(1, 2, 0, 3).reshape(P, DC * FCL * P))
            for e in range(E)])
        # b1p[p][e*FCL+fc] = b1[e][lo + fc*128 + p]
        b1p = np.stack([
            b1[e][lo:hi].reshape(FCL, P).T for e in range(E)],
            axis=1).reshape(P, E * FCL)
        in_maps.append({
            "xT": xT,
            "w1p": np.ascontiguousarray(w1p.astype(bf16)),
            "w2p": np.ascontiguousarray(w2p.astype(bf16)),
            "b1p": np.ascontiguousarray(b1p.astype(np.float32)),
        })
    return in_maps, pes


def kernel(x, gate_w, gate_b, w1, b1, w2, b2, _run_kwargs=None, _out=None):
    x = np.asarray(x, np.float32)
    gate_w = np.asarray(gate_w, np.float32)
    gate_b = np.asarray(gate_b, np.float32)
    w1 = np.asarray(w1, np.float32)
    b1 = np.asarray(b1, np.float32)
    w2 = np.asarray(w2, np.float32)
    b2 = np.asarray(b2, np.float32)

    x2d = x.reshape(-1, D)
    idx, cw = _route(x2d, gate_w, gate_b)
    in_maps, pes = _pack_inputs(x2d, idx, w1, b1, w2)
    key = tuple(pes)
    if key not in _program_cache:
        _program_cache[key] = _build_program(pes)
    nc = _program_cache[key]

    try:
        res = run_bass_kernel_spmd(
            nc, in_maps, core_ids=list(range(E)), **(_run_kwargs or {})
        )
    except Exception:
        # transient device states (e.g. NRT_EXEC_UNIT_UNRECOVERABLE) usually
        # clear on retry
        import time as _time
        _time.sleep(10)
        res = run_bass_kernel_spmd(
            nc, in_maps, core_ids=list(range(E)), **(_run_kwargs or {})
        )
    if _out is not None:
        _out.append(res)

    capt = sum(pes)
    ysum = np.zeros((capt, D), np.float32)
    for c in range(E):
        # yT [DC, P, capt] -> y [capt, D]
        ysum += res.results[c]["yT"].astype(np.float32) \
            .transpose(2, 0, 1).reshape(capt, D)

    out = np.zeros((B * S, D), np.float32)
    oe = 0
    for e in range(E):
        n_e = len(idx[e])
        out[idx[e]] += cw[e][:, None] * (ysum[oe:oe + n_e] + b2[e])
        oe += pes[e]
    return out.reshape(B, S, D)

